# revision 10
# baseline (speedup 1.0000x reference)
"""DiscreteHMM log-likelihood on 8 Trainium2 NeuronCores.

Math: the reference forward algorithm in log space is computed in scaled
probability space (classic scaled forward algorithm):
    p_{t+1} = (p_t @ Aq) * E_{t+1}
where Aq ~ 1024*A is a RANK-128 factorization Aq = W1 @ W2 held in fp8e4
(TRN E4M3, max 240):
  - A = softmax(log_A, rows); SVD -> W1 = U sqrt(S), W2 = sqrt(S) V^T
    (balanced per-component scales), W1 scaled by s1 and quantized to fp8;
    W2 refit by least squares against s1*A to cancel W1's quantization
    error, scaled by s2, quantized.  E stream carries 1024/(s1*s2) so the
    net per-step factor is 1024 (measured end-to-end rel err ~7e-5 vs the
    f64 reference, tolerance 2e-2).
Final per-sequence loglik = ln(sum_j p_T) - T*ln(1024).

Sharding: data-parallel over batch -- 8 sequences per core, parameters
replicated; per-sequence logliks summed on host.

Device schedule (the whole point): the scan is LDWEIGHTS-bound on the PE
(128-col fp8 FWL load = ~26.6 ns).  Rank 128 means only 8 weight loads
per timestep (4x W1 chunk + 4x W2 chunk) instead of 16 for full A.  The
per-step serial latency loop (MM drain -> sem -> ScalarE y-cast -> sem ->
MM -> drain -> sem -> DVE emission-mult -> sem) is ~1.2 us, far more than
the 213 ns of PE work per step -- so the 8 sequences run as 8 INDEPENDENT
phase-shifted chains (moving free dim = 1), one PSUM bank each.  Per round
(one timestep for all 8 chains) the PE does 64 LDW+MM pairs ~ 1.7 us,
which fully hides every chain's latency loop.

Per chain-step:
  s1:  y = W1^T p      4 accumulating matmuls into psum[:, j, 0:1]
  cast: ScalarE copies y (psum f32) -> SBUF bf16  (PE cannot read PSUM)
  s2:  out[m] = W2m^T y  4 single matmuls into psum[:, j, 1+m]
  emult: DVE multiplies out by the emission column -> next p (bf16)
Chains are paired two-per-psum-tile (tile = (128, 2, 512) f32 = 2 banks,
chain j owns bank j) so one DVE op covers two chains -- DVE op cost is
overhead-dominated (~174 ns), and 8 single-chain ops per round would lag
the round period.
"""

import numpy as np
import ml_dtypes
from contextlib import ExitStack

import concourse.bass as bass
import concourse.bacc as bacc
import concourse.mybir as mybir
import concourse.tile as tile
from concourse.bass_utils import run_bass_kernel_spmd

S = 512          # states
O = 1024         # observation symbols
B = 64           # batch
T = 512          # timesteps
NCORES = 8
P = 128          # partition size
KC = S // P      # 4 state chunks
R = 128          # factorization rank
G = 8            # independent chains per core (1 sequence each)
NQ = G // 2      # chain pairs (psum tiles)
W = G * KC       # 32: emission elems per step per partition
TBLK = 64        # timesteps per emission DMA block
NBLK = T // TBLK
E0SPLIT = 16     # block 0 arrives as 4 slices of 16 steps for an early start

F32 = mybir.dt.float32
BF16 = mybir.dt.bfloat16
F8 = mybir.dt.float8e4
_BF16_NP = ml_dtypes.bfloat16
_F8_NP = ml_dtypes.float8_e4m3

_cached_nc = None


def _build_nc() -> bass.Bass:
    nc = bacc.Bacc()
    w1_d = nc.dram_tensor("w1_mat", (S, R), F8, kind="ExternalInput")
    w2_d = nc.dram_tensor("w2_mat", (R, S), F8, kind="ExternalInput")
    pi_d = nc.dram_tensor("pi_vec", (P, G), F32, kind="ExternalInput")
    e_d = nc.dram_tensor("e_str", (NBLK, P, TBLK * W), F32, kind="ExternalInput")
    out_d = nc.dram_tensor("out_ll", (1, G), F32, kind="ExternalOutput")

    with ExitStack() as ctx:
        tc = ctx.enter_context(tile.TileContext(nc))
        const = ctx.enter_context(tc.tile_pool(name="const", bufs=1))
        epool = ctx.enter_context(tc.tile_pool(name="epool", bufs=2))
        pspool = ctx.enter_context(tc.tile_pool(name="psum", bufs=1, space="PSUM"))

        # prologue DMAs, issued in consumption order: pi + the first block-0
        # emission slice (init), W1 chunks (round 1 s1), the rest.
        pi_t = const.tile([P, G], F32, name="pi", tag="pi")
        nc.sync.dma_start(pi_t[:], pi_d[:, :])
        e0q = []
        for i in range(4):
            e0q.append(const.tile([P, E0SPLIT * W], F32, name=f"e0q{i}", tag=f"e0q{i}"))
        nc.sync.dma_start(e0q[0][:], e_d[0][:, 0:E0SPLIT * W])
        w1k = []
        for k in range(KC):
            w1k.append(const.tile([P, R], F8, name=f"w1_{k}", tag=f"w1_{k}"))
            nc.sync.dma_start(w1k[k][:], w1_d[k * P:(k + 1) * P, :])
        nc.sync.dma_start(e0q[1][:], e_d[0][:, E0SPLIT * W:2 * E0SPLIT * W])
        w2m = []
        for m in range(KC):
            w2m.append(const.tile([P, P], F8, name=f"w2_{m}", tag=f"w2_{m}"))
            nc.sync.dma_start(w2m[m][:], w2_d[:, m * P:(m + 1) * P])
        for i in (2, 3):
            nc.sync.dma_start(e0q[i][:],
                              e_d[0][:, i * E0SPLIT * W:(i + 1) * E0SPLIT * W])
        ones_t = const.tile([P, 1], BF16, name="ones", tag="ones")
        nc.vector.memset(ones_t[:], 1.0)

        # persistent per-pair state:
        #   psum tile (P, 2, 512) f32 = 2 banks; chain j owns bank j:
        #     y at [:, j, 0:1], out chunks at [:, j, 1:5], mass at [0:1, j, 0:1]
        #   p tile (P, 2, KC) bf16: p chunk k of chain j at [:, j, k]
        ps = [pspool.tile([P, 2, 512], F32, name=f"ps{q}", tag=f"ps{q}")
              for q in range(NQ)]
        pp = [const.tile([P, 2, KC], BF16, name=f"p{q}", tag=f"p{q}")
              for q in range(NQ)]
        yc = [const.tile([P, 2], BF16, name=f"y{q}", tag=f"y{q}")
              for q in range(NQ)]

        # init: p0 = pi * E0 (E0 carries the 1024), one DVE op per pair
        pi3 = pi_t[:].rearrange("p (x k) -> p x k", k=KC)
        for q in range(NQ):
            esl = e0q[0][:, 4 * (2 * q):4 * (2 * q) + 8]
            nc.vector.tensor_mul(pp[q][:], esl.rearrange("p (x k) -> p x k", k=KC),
                                 pi3)

        def e_slice(t, q):
            # (P, 2, KC) emission slice for pair q at timestep t
            if t < TBLK:
                src, tt = e0q[t // E0SPLIT], t % E0SPLIT
            else:
                src, tt = eb, t % TBLK
            ap = src[:, tt * W + 4 * (2 * q): tt * W + 4 * (2 * q) + 8]
            return ap.rearrange("p (x k) -> p x k", k=KC)

        def emit_s1(t):
            # stage 1 for all chains: y_g = W1^T p_g (4 accumulating MMs),
            # then the paired y casts on DVE (after the emults in priority).
            for g in range(G):
                q, j = g // 2, g % 2
                for k in range(KC):
                    nc.tensor.matmul(ps[q][:, j, 0:1], w1k[k][:],
                                     pp[q][:, j, k:k + 1],
                                     start=(k == 0), stop=(k == KC - 1),
                                     skip_group_check=True)
            for q in range(NQ):
                nc.vector.tensor_copy(
                    yc[q][:], ps[q][:, :, 0:1].rearrange("p x o -> p (x o)"))

        def emit_s2_em(t):
            # stage 2 for all chains (out_m = W2m^T y, single MMs), then the
            # paired emission multiplies -> p_t (bf16)
            for g in range(G):
                q, j = g // 2, g % 2
                for m in range(KC):
                    nc.tensor.matmul(ps[q][:, j, 1 + m:2 + m], w2m[m][:],
                                     yc[q][:, j:j + 1],
                                     start=False, stop=(m == KC - 1),
                                     skip_group_check=True)
            for q in range(NQ):
                nc.vector.tensor_mul(pp[q][:], ps[q][:, :, 1:5], e_slice(t, q))

        # Software-pipelined emission: round block for step t emits
        # [s2+emult of step t-1] then [s1+cast of step t].  The per-engine
        # program order this pins gives every chain's drain->sem->cast->sem
        # chain most of a round of slack before its s2 runs.
        eb = eb_next = None
        emit_s1(1)
        for t in range(2, T):
            tp = t - 1        # emult(t-1) consumes emission column t-1
            if tp % TBLK == 8 and tp // TBLK < NBLK - 1:
                eb_next = epool.tile([P, TBLK * W], F32, name="eb", tag="eb")
                nc.sync.dma_start(eb_next[:], e_d[tp // TBLK + 1])
            if tp >= TBLK and tp % TBLK == 0:
                eb = eb_next
            emit_s2_em(tp)
            emit_s1(t)
        emit_s2_em(T - 1)

        # final masses + ln
        lls = const.tile([1, G], F32, name="ll", tag="ll")
        for g in range(G):
            q, j = g // 2, g % 2
            for k in range(KC):
                nc.tensor.matmul(ps[q][0:1, j, 0:1], ones_t[:],
                                 pp[q][:, j, k:k + 1],
                                 start=(k == 0), stop=(k == KC - 1),
                                 skip_group_check=True)
        for g in range(G):
            q, j = g // 2, g % 2
            nc.scalar.activation(lls[0:1, g:g + 1], ps[q][0:1, j, 0:1],
                                 mybir.ActivationFunctionType.Ln)
        nc.sync.dma_start(out_d[:, :], lls[:])
    nc.finalize()
    return nc


def _softmax(x, axis):
    x = x - x.max(axis=axis, keepdims=True)
    e = np.exp(x)
    return e / e.sum(axis=axis, keepdims=True)


def _factorize(A):
    """A (f64, rows sum to 1) -> (W1q fp8 (S,R), W2q fp8 (R,S), escale) with
    W1q @ W2q ~ (1024/escale) * A."""
    U, sv, Vt = np.linalg.svd(A)
    W1 = U[:, :R] * np.sqrt(sv[:R])
    W2 = np.sqrt(sv[:R])[:, None] * Vt[:R, :]
    c1 = np.abs(W1).max(axis=0)
    c2 = np.abs(W2).max(axis=1)
    dbal = np.sqrt(c2 / c1)
    W1 *= dbal
    s1 = 2.0 ** np.floor(np.log2(224.0 / np.abs(W1).max()))
    W1q = (W1 * s1).astype(_F8_NP)
    W2f = np.linalg.lstsq(W1q.astype(np.float64), s1 * A, rcond=None)[0]
    s2 = 2.0 ** np.floor(np.log2(224.0 / np.abs(W2f).max()))
    W2q = (W2f * s2).astype(_F8_NP)
    return W1q, W2q, np.float64(O) / (s1 * s2)


def kernel(observations, log_pi, log_A, log_B):
    global _cached_nc
    obs = np.asarray(observations)
    A = _softmax(np.asarray(log_A, dtype=np.float64), 1)
    Bp = _softmax(np.asarray(log_B, dtype=np.float64), 1)
    pi = _softmax(np.asarray(log_pi, dtype=np.float64), 0).astype(np.float32)

    W1q, W2q, escale = _factorize(A)
    pi_in = np.tile(np.ascontiguousarray(pi.reshape(KC, P).T), (1, 2))  # (P, 8)=2x(p,k)
    # X[j, b, t] = escale * B[j, o_{b,t}]  (t=0 column: 1024 * B, multiplies pi)
    X = (Bp[:, obs] * escale).astype(np.float32)
    X[:, :, 0] = (np.float64(O) * Bp[:, obs[:, 0]]).astype(np.float32)

    in_maps = []
    for c in range(NCORES):
        xc = X[:, c * G:(c + 1) * G, :]                      # (S, G, T)
        ec = xc.reshape(KC, P, G, NBLK, TBLK)                # (k, p, g, blk, tt)
        ec = np.ascontiguousarray(ec.transpose(3, 1, 4, 2, 0))  # (blk, p, tt, g, k)
        in_maps.append({
            "w1_mat": W1q,
            "w2_mat": W2q,
            "pi_vec": pi_in,
            "e_str": ec.reshape(NBLK, P, TBLK * W),
        })

    if _cached_nc is None:
        _cached_nc = _build_nc()
    res = run_bass_kernel_spmd(_cached_nc, in_maps, list(range(NCORES)))
    lls = np.concatenate([res.results[c]["out_ll"][0] for c in range(NCORES)])
    total = np.float64(lls.sum()) - np.float64(B) * T * np.log(np.float64(O))
    return np.asarray(np.float32(total))


# revision 12
# speedup vs baseline: 2.5742x; 2.5742x over previous
"""DiscreteHMM log-likelihood on 8 Trainium2 NeuronCores.

Math: the reference forward algorithm in log space,
    alpha_{t+1,j} = logsumexp_i(alpha_{t,i} + lA[i,j]) + lB[j, o_{t+1}]
is computed here in *probability* space (classic scaled forward algorithm):
    p_{t+1} = (p_t @ A) * E_{t+1},   A = softmax(log_A, rows), E_t = 1024*B[:, o_t]
The transition preserves total mass (A rows sum to 1) and the emission
multiply scales it by ~1/1024 on average (column means of a softmax row-
normalized 512x1024 table), so with the constant 1024 rescale folded into E
the running mass drifts only a few nats over all 512 steps (measured
[-4.1, +3.5] for these inputs) -- no per-step renormalization is needed.
Final per-sequence loglik = ln(sum_j p_T) - T*ln(1024).

Sharding: data-parallel over batch -- 8 sequences per core, parameters
replicated; per-sequence logliks are summed on host (64 adds).

Device layout (states-major): p is a (512 states x 8 batch) column block,
packed as ONE SBUF tile of (128, 32) bf16 -- column block m holds state
chunk j in [128m, 128m+128). Each step: 16 matmuls
psum[:, 8m:8m+8] += A[128k:,128m:].T @ p[:, 8k:8k+8] (A chunks stationary
128x128 bf16 weights, batch the 8-wide moving operand), then ONE DVE
multiply with the pre-gathered emission tile (128, 32) -> next p.
Emissions are gathered on host into a per-core stream with matching
(p, t, m, b) layout and double-buffered into SBUF in 64-step blocks.
"""

import numpy as np
import ml_dtypes
from contextlib import ExitStack

import concourse.bass as bass
import concourse.bacc as bacc
import concourse.mybir as mybir
import concourse.tile as tile
from concourse.bass_utils import run_bass_kernel_spmd

S = 512          # states
O = 1024         # observation symbols
B = 64           # batch
T = 512          # timesteps
NCORES = 8
BSH = B // NCORES          # sequences per core
P = 128                    # partition size
KC = S // P                # 4 state chunks
W = KC * BSH               # 32: packed free width of the p tile
TBLK = 64                  # timesteps per emission DMA block
NBLK = T // TBLK

F32 = mybir.dt.float32
BF16 = mybir.dt.bfloat16
F8 = mybir.dt.float8e4
_BF16_NP = ml_dtypes.bfloat16
_F8_NP = ml_dtypes.float8_e4m3
ASCALE = 1024.0   # host folds x1024 into fp8 A; emission stream drops its 1024

_cached_nc = None


def _build_nc() -> bass.Bass:
    nc = bacc.Bacc()
    a_d = nc.dram_tensor("a_mat", (S, S), F8, kind="ExternalInput")
    pi_d = nc.dram_tensor("pi_vec", (P, KC), F32, kind="ExternalInput")
    e_d = nc.dram_tensor("e_str", (NBLK, P, TBLK * W), F32, kind="ExternalInput")
    out_d = nc.dram_tensor("out_ll", (1, BSH), F32, kind="ExternalOutput")

    with ExitStack() as ctx:
        tc = ctx.enter_context(tile.TileContext(nc))
        const = ctx.enter_context(tc.tile_pool(name="const", bufs=1))
        epool = ctx.enter_context(tc.tile_pool(name="epool", bufs=2))
        ppool = ctx.enter_context(tc.tile_pool(name="ppool", bufs=5))
        pspool = ctx.enter_context(tc.tile_pool(name="psum", bufs=2, space="PSUM"))

        # prologue DMAs: one per A row-chunk (ordered by first use), one for
        # pi, and block-0 emissions as 4 quarter tiles so all transfers run
        # on parallel HWDGE queues.
        pi_t = const.tile([P, KC], F32, name="pi", tag="pi")
        nc.sync.dma_start(pi_t[:], pi_d[:, :])
        # block-0 emissions in uneven slices (first slice small so the scan
        # starts early); A chunks ordered by first use; late e slices issued
        # last so no early consumer shares a DMA-queue sem with them.
        E0SPLIT = (8, 8, 16, 32)
        E0OFF = (0, 8, 16, 32)
        e0q = []
        t_off = 0
        for i, n in enumerate(E0SPLIT):
            e0q.append(const.tile([P, n * W], F32, name=f"e0q{i}", tag=f"e0q{i}"))
        nc.sync.dma_start(e0q[0][:], e_d[0][:, 0:E0SPLIT[0] * W])
        a_t = {}
        for k in (2, 3, 0, 1):
            a_t[k] = const.tile([P, S], F8, name=f"a{k}", tag=f"a{k}")
            nc.sync.dma_start(a_t[k][:], a_d[k * P:(k + 1) * P, :])
        for i in (1, 2, 3):
            nc.sync.dma_start(e0q[i][:],
                              e_d[0][:, E0OFF[i] * W:(E0OFF[i] + E0SPLIT[i]) * W])
        ones_t = const.tile([P, 1], BF16, name="ones", tag="ones")
        nc.vector.memset(ones_t[:], 1.0)

        def load_eblk(blk):
            et = epool.tile([P, TBLK * W], F32, name="eb", tag="eb")
            nc.sync.dma_start(et[:], e_d[blk])
            return et

        eb = None
        # p is held as two packed half tiles: pA = chunks {0,1}, pB = {2,3};
        # 3D (P, 2, BSH) so the DVE multiply covers both chunks in one op.
        pA = ppool.tile([P, 2, BSH], BF16, name="pA", tag="pA")
        pB = ppool.tile([P, 2, BSH], BF16, name="pB", tag="pB")
        for m in range(KC):
            dst = pA if m < 2 else pB
            nc.vector.tensor_scalar_mul(dst[:, m % 2, :],
                                        e0q[0][:, m * BSH:(m + 1) * BSH],
                                        pi_t[:, m:m + 1])

        def p_slice(k):
            src = pA if k < 2 else pB
            return src[:, k % 2, :]

        def e_slice(src_t, tt, half):
            ap = src_t[:, tt * W + half * 2 * BSH: tt * W + (half + 1) * 2 * BSH]
            return ap.rearrange("p (x b) -> p x b", b=BSH)

        # Matmul slot order + paired DVE multiplies chosen by simulating the
        # steady-state latency loop (MM drain -> sem -> DVE -> sem -> MM):
        # groups m2/m3 complete early and feed the first DVE op; their
        # chunks are consumed late in the next step. Accumulation groups
        # interleave, so each pair member gets its own PSUM bank: the pair
        # psum tile is (P, 2, 512) f32 = two banks, chunk m at [:, m%2, 0:8].
        # fp8 slot order: psB (m2,m3) finishes by slot 9 and feeds DVE op 1;
        # k0/k1 operands (from the previous step's late DVE op) are not
        # consumed before slot 6.
        SLOTS = [(2, 3), (3, 3), (2, 2), (3, 2), (0, 3), (1, 3), (2, 1), (3, 1),
                 (2, 0), (3, 0), (0, 2), (1, 2), (0, 1), (1, 1), (0, 0), (1, 0)]
        for blk in range(NBLK):
            if blk > 0:
                eb = load_eblk(blk)
            for tt in range(1 if blk == 0 else 0, TBLK):
                psA = pspool.tile([P, 2, 512], F32, name="psA", tag="psA")
                psB = pspool.tile([P, 2, 512], F32, name="psB", tag="psB")
                done = [0] * KC
                for (m, k) in SLOTS:
                    dst = psA if m < 2 else psB
                    done[m] += 1
                    nc.tensor.matmul(dst[:, m % 2, 0:BSH],
                                     a_t[k][:, m * P:(m + 1) * P], p_slice(k),
                                     start=(done[m] == 1), stop=(done[m] == KC),
                                     skip_group_check=True)
                if blk == 0:
                    qi = 0 if tt < 8 else (1 if tt < 16 else (2 if tt < 32 else 3))
                    esrc, ett = e0q[qi], tt - E0OFF[qi]
                else:
                    esrc, ett = eb, tt
                pB = ppool.tile([P, 2, BSH], BF16, name="pB", tag="pB")
                nc.vector.tensor_mul(pB[:], psB[:, :, 0:BSH], e_slice(esrc, ett, 1))
                pA = ppool.tile([P, 2, BSH], BF16, name="pA", tag="pA")
                nc.vector.tensor_mul(pA[:], psA[:, :, 0:BSH], e_slice(esrc, ett, 0))

        msum = pspool.tile([1, BSH], F32, name="msum", tag="psA")
        for k in range(KC):
            nc.tensor.matmul(msum[:], ones_t[:], p_slice(k),
                             start=(k == 0), stop=(k == KC - 1))
        lls = const.tile([1, BSH], F32, name="ll", tag="ll")
        nc.scalar.activation(lls[:], msum[:], mybir.ActivationFunctionType.Ln)
        nc.sync.dma_start(out_d[:, :], lls[:])
    nc.finalize()
    return nc


def _softmax(x, axis):
    x = x - x.max(axis=axis, keepdims=True)
    e = np.exp(x)
    return e / e.sum(axis=axis, keepdims=True)


def kernel(observations, log_pi, log_A, log_B):
    global _cached_nc
    obs = np.asarray(observations)
    A = _softmax(np.asarray(log_A, dtype=np.float64), 1)
    Bp = _softmax(np.asarray(log_B, dtype=np.float64), 1).astype(np.float32)
    pi = _softmax(np.asarray(log_pi, dtype=np.float64), 0).astype(np.float32)

    a_bf = (A * ASCALE).astype(_F8_NP)
    pi_in = np.ascontiguousarray(pi.reshape(KC, P).T)
    # X[j, b, t] = B[j, o_{b,t}]; the per-step 1024 lives in fp8 A.
    X = Bp[:, obs].astype(np.float32)
    X[:, :, 0] *= np.float32(O)

    in_maps = []
    for c in range(NCORES):
        xc = X[:, c * BSH:(c + 1) * BSH, :]                    # (S, BSH, T)
        ec = xc.reshape(KC, P, BSH, NBLK, TBLK)                # (m, p, b, blk, t')
        ec = np.ascontiguousarray(ec.transpose(3, 1, 4, 0, 2))  # (blk, p, t', m, b)
        in_maps.append({
            "a_mat": a_bf,
            "pi_vec": pi_in,
            "e_str": ec.reshape(NBLK, P, TBLK * W),
        })

    if _cached_nc is None:
        _cached_nc = _build_nc()
    res = run_bass_kernel_spmd(_cached_nc, in_maps, list(range(NCORES)))
    lls = np.concatenate([res.results[c]["out_ll"][0] for c in range(NCORES)])
    total = np.float64(lls.sum()) - np.float64(B) * T * np.log(np.float64(O))
    return np.asarray(np.float32(total))

